# revision 1
# baseline (speedup 1.0000x reference)
"""Trainium2 Bass kernel for a 2-step BasicNCA2D cell update.

Strategy
--------
Data-parallel over batch: 8 images, one per NeuronCore. Per core the two NCA
steps are fused on-chip (x never round-trips to DRAM between steps).

Per step the math is
    y  = depthwise_conv5x5(x, conv_w) + conv_b        (reflect padding)
    h  = relu([x, y] @ fc0_w + fc0_b)
    dx = h @ fc1_w
    x' = concat([x[..., :1], x[..., 1:] + dx])

conv+fc0 are fused into a bank of accumulating matmuls:
    h_pre = sum_{di,dj} x_shift(di,dj) @ M[di,dj],
    M[di,dj] = diag(conv_w[di,dj]) @ fc0_w[24:] (+ fc0_w[:24] at center)

Rows are processed in groups of 4. Channels are zero-padded 24->32 host-side
so every partition split lands on the hardware-legal bases {0,32,64,96}.
Inputs are staged in SBUF "v-blocks": block k = image rows 4k-2..4k+1 at
partitions (g*32 + c), with 2 reflect-halo columns per side (width 516).
An output group (rows 4m..4m+3) reads exactly blocks m and m+1, so conv+fc0
for 4 rows x 512 cols is 10 matmuls (5 horizontal shifts x 2 blocks) with
K=128, M=128=(4 rows x 32 hidden), N=512, accumulated in one PSUM bank.
Vertical taps ride in the partition stacking; horizontal taps are free-dim
offsets into the 516-wide block. Matmuls run in float32r (full-rate PE).

relu+bias on ScalarE, fc1 as one K=128 matmul whose output partitions are
pre-arranged as (row, channel) with zero columns at channel 0 and at the
pads, so the DVE residual add 'psum + x' lands channel 0 = x[...,0] and
pad channels = 0 for free. Reflect-halo columns of intermediate tiles are
filled by GPSIMD copies in the same writer phase (no write-after-read
hazards on block tiles, which would serialize the PE stream).

Groups are emitted in pairs per software-pipeline iteration so each
stage's conv burst (~4.3us of PE work) covers the other stage's
relu->fc1->residual latency chain; fc1+residual trail their conv group by
one iteration. Measured ~790us/core/pass on HW (cost model: 670us; PE
busy floor ~610us).
"""

import numpy as np

import concourse.mybir as mybir
import concourse.tile as tile
from concourse import bacc
from concourse.bass_utils import run_bass_kernel_spmd

F32 = mybir.dt.float32
F32R = mybir.dt.float32r

H = 512
W = 512
C = 24
CP = 32  # padded channels
HD = 32
NCORES = 8
NBLK = H // 4 + 1  # 129 input v-blocks per stage


def _build_nc(steps: int, repeat: int = 1):
    nc = bacc.Bacc("TRN2", target_bir_lowering=False, debug=False)

    X = nc.dram_tensor("X", [CP, H + 4, W + 4], F32R, kind="ExternalInput")
    WAB = nc.dram_tensor("WAB", [2, 5, 128, 128], F32R, kind="ExternalInput")
    WC = nc.dram_tensor("WC", [128, 128], F32R, kind="ExternalInput")
    BIAS = nc.dram_tensor("BIAS", [128, 1], F32, kind="ExternalInput")
    Y = nc.dram_tensor("Y", [CP, H, W], F32, kind="ExternalOutput")

    with tile.TileContext(nc) as tc:
        with (
            tc.tile_pool(name="wpool", bufs=1) as wpool,
            tc.tile_pool(name="xpool", bufs=12) as xpool,
            tc.tile_pool(name="hpool", bufs=8) as hpool,
            tc.tile_pool(name="opool", bufs=5) as opool,
            tc.tile_pool(name="pp", bufs=2, space="PSUM") as pp,
            tc.tile_pool(name="ppdx", bufs=2, space="PSUM") as ppdx,
        ):
            # ---- weights ----
            wab_t = wpool.tile([128, 2, 5, 128], F32R, tag="wab")
            nc.sync.dma_start(wab_t[:], WAB.ap().transpose([2, 0, 1, 3]))
            wc_t = wpool.tile([128, 128], F32R, tag="wc")
            nc.sync.dma_start(wc_t[:], WC.ap())
            bias_t = wpool.tile([128, 1], F32, tag="bias")
            nc.sync.dma_start(bias_t[:], BIAS.ap())

            # per-stage block tiles, keyed [stage][block]
            blocks = []


            def load_x0_block(k):
                # X is reflect-padded host-side: padded row/col i = image i-2.
                t = xpool.tile([128, 516], F32R, tag="x0", name=f"x0_{k}")
                blocks[0][k] = t
                nc.sync.dma_start(
                    t[:],
                    X.ap()[:, 4 * k : 4 * k + 4, :].transpose([1, 0, 2]),
                )

            pend = [dict() for _ in range(steps)]

            def stage_part1(s, g):
                """Conv+fc0 matmuls and relu for stage s, output rows 4g..4g+3."""
                blk_a = blocks[s][g]
                blk_b = blocks[s][g + 1]
                hp = pp.tile([128, 512], F32, tag=f"hp{s}", name=f"hp{s}_{g}")
                for dj in range(5):
                    nc.tensor.matmul(
                        hp[:],
                        wab_t[:, 0, dj, :],
                        blk_a[:, dj : dj + 512],
                        start=(dj == 0),
                        stop=False,
                    )
                    nc.tensor.matmul(
                        hp[:],
                        wab_t[:, 1, dj, :],
                        blk_b[:, dj : dj + 512],
                        start=False,
                        stop=(dj == 4),
                    )
                h = hpool.tile([128, 512], F32R, tag=f"h{s}", name=f"h{s}_{g}")
                nc.scalar.activation(
                    h[:], hp[:], mybir.ActivationFunctionType.Relu, bias=bias_t[:]
                )
                pend[s][g] = h

            def stage_part2(s, g):
                """fc1 + residual for stage s group g (one iteration later)."""
                last = s == steps - 1
                blk_a = blocks[s][g]
                blk_b = blocks[s][g + 1]
                h = pend[s].pop(g)
                dxp = ppdx.tile([128, 512], F32, tag=f"dx{s}", name=f"dx{s}_{g}")
                nc.tensor.matmul(dxp[:], wc_t[:], h[:], start=True, stop=True)

                if last:
                    out = opool.tile([128, 512], F32, tag="out", name=f"out_{g}")
                    nc.vector.tensor_add(
                        out[0:64, :], dxp[0:64, :].bitcast(F32R), blk_a[64:128, 2:514]
                    )
                    nc.vector.tensor_add(
                        out[64:128, :], dxp[64:128, :].bitcast(F32R), blk_b[0:64, 2:514]
                    )
                    nc.sync.dma_start(
                        Y.ap()[:, 4 * g : 4 * g + 4, :].transpose([1, 0, 2]),
                        out[:],
                    )
                    return

                # intermediate stage: write into next stage's block tiles
                nxt = blocks[s + 1]
                if g == 0:
                    nxt[0] = xpool.tile(
                        [128, 516], F32R, tag=f"x{s+1}", name=f"x{s+1}_0"
                    )
                if g + 1 not in nxt:
                    nxt[g + 1] = xpool.tile(
                        [128, 516], F32R, tag=f"x{s+1}", name=f"x{s+1}_{g+1}"
                    )
                na, nb = nxt[g], nxt[g + 1]
                for lo, hi, dst, blk in ((0, 64, "hi", blk_a), (64, 128, "lo", blk_b)):
                    t = na if dst == "hi" else nb
                    tl, th = (64, 128) if dst == "hi" else (0, 64)
                    nc.vector.tensor_add(
                        t[tl:th, 2:514], dxp[lo:hi, :].bitcast(F32R), blk[tl:th, 2:514]
                    )
                    # reflect-halo columns copied from the freshly written cols
                    for vc, pc in ((0, 4), (1, 3), (514, 512), (515, 511)):
                        nc.gpsimd.tensor_copy(
                            t[tl:th, vc : vc + 1], t[tl:th, pc : pc + 1]
                        )
                if g == 0:
                    # top reflect rows: blk0 g0 <- row 2 (= blk1 g0), g1 <- row 1 (= blk0 g3)
                    nc.gpsimd.tensor_copy(na[0:32, :], nb[0:32, :])
                    nc.gpsimd.tensor_copy(na[32:64, :], na[96:128, :])
                if g == H // 4 - 1:
                    # bottom block (g+1): g2 <- row 510 (= its g0), g3 <- row 509 (= blk g's g3)
                    nc.gpsimd.tensor_copy(nb[64:96, :], nb[0:32, :])
                    nc.gpsimd.tensor_copy(nb[96:128, :], na[96:128, :])

            n_pairs = H // 8
            n_iters = n_pairs + 3 * steps + 3
            for _rep in range(repeat):
                blocks.clear()
                blocks.extend(dict() for _ in range(steps))
                for m in range(n_iters + 1):
                    for k in (2 * m, 2 * m + 1):
                        if k < NBLK:
                            load_x0_block(k)
                    for s in range(steps):
                        p = m - 1 - 3 * s
                        if 0 <= p < n_pairs:
                            stage_part1(s, 2 * p)
                            stage_part1(s, 2 * p + 1)
                    for s in range(steps):
                        p2 = m - 2 - 3 * s
                        if 0 <= p2 < n_pairs:
                            stage_part2(s, 2 * p2)
                            stage_part2(s, 2 * p2 + 1)

    nc.compile()
    return nc


_NC_CACHE = {}
_REPEAT = 1


def _get_nc(steps):
    key = (steps, _REPEAT)
    if key not in _NC_CACHE:
        _NC_CACHE[key] = _build_nc(steps, repeat=_REPEAT)
    return _NC_CACHE[key]


def _prep_weights(conv_w, conv_b, fc0_w, fc0_b, fc1_w):
    conv_w = np.asarray(conv_w, np.float64)[:, :, 0, :]  # [5,5,24]
    W1 = np.asarray(fc0_w, np.float64)[:C]  # [24,32]
    W2 = np.asarray(fc0_w, np.float64)[C:]  # [24,32]
    fc1_w = np.asarray(fc1_w, np.float64)  # [32,23]

    # M[ki, kj] = diag(conv_w[ki,kj]) @ W2 (+ W1 at center)
    M = conv_w[:, :, :, None] * W2[None, None, :, :]  # [5,5,24,32]
    M[2, 2] += W1

    WAB = np.zeros((2, 5, 128, 128), np.float32)
    for dj in range(5):
        for g in range(4):
            for f in range(4):
                ka = g - f  # di+2 for block A (di = g-2-f)
                if g >= f and 0 <= ka <= 4:
                    WAB[0, dj, g * 32 : g * 32 + C, f * 32 : f * 32 + HD] = M[ka, dj]
                kb = g + 4 - f  # di+2 for block B (di = g+2-f)
                if g <= f and 0 <= kb <= 4:
                    WAB[1, dj, g * 32 : g * 32 + C, f * 32 : f * 32 + HD] = M[kb, dj]

    WC = np.zeros((128, 128), np.float32)
    for f in range(4):
        WC[f * 32 : f * 32 + HD, f * 32 + 1 : f * 32 + C] = fc1_w

    bias_eff = (
        np.asarray(fc0_b, np.float64) + np.asarray(conv_b, np.float64) @ W2
    ).astype(np.float32)
    BIAS = np.tile(bias_eff, 4).reshape(128, 1)
    return WAB, WC, BIAS


def _run_pass(x_chw, WAB, WC, BIAS, steps):
    """One device invocation: `steps` NCA steps on x [B, C, H, W] fp32."""
    B = x_chw.shape[0]
    x_t = np.zeros((B, CP, H + 4, W + 4), np.float32)
    x_t[:, :C] = np.pad(x_chw, ((0, 0), (0, 0), (2, 2), (2, 2)), mode="reflect")
    nc = _get_nc(steps)
    in_maps = [
        {"X": x_t[i % B], "WAB": WAB, "WC": WC, "BIAS": BIAS} for i in range(NCORES)
    ]
    res = run_bass_kernel_spmd(nc, in_maps, core_ids=list(range(NCORES)))
    globals()["LAST_RESULTS"] = res
    return np.stack([res.results[i]["Y"][:C] for i in range(B)])  # [B, C, H, W]


def kernel(x, conv_w, conv_b, fc0_w, fc0_b, fc1_w, steps):
    steps = int(steps)
    x = np.asarray(x, np.float32)
    B = x.shape[0]
    assert x.shape == (B, H, W, C) and 1 <= B <= NCORES, x.shape
    if steps <= 0:
        return x.copy()

    WAB, WC, BIAS = _prep_weights(conv_w, conv_b, fc0_w, fc0_b, fc1_w)
    x_chw = np.ascontiguousarray(x.transpose(0, 3, 1, 2))
    # device pipeline supports 2 fused steps; decompose larger step counts
    while steps > 0:
        n = 2 if steps >= 2 else 1
        x_chw = _run_pass(x_chw, WAB, WC, BIAS, n)
        steps -= n
    return np.ascontiguousarray(x_chw.transpose(0, 2, 3, 1)).astype(np.float32)


if __name__ == "__main__":
    rng = np.random.default_rng(0)
    inputs = {
        "x": rng.standard_normal((8, H, W, C), dtype=np.float32),
        "conv_w": (rng.standard_normal((5, 5, 1, C)) * 0.1).astype(np.float32),
        "conv_b": (rng.standard_normal((C,)) * 0.1).astype(np.float32),
        "fc0_w": (rng.standard_normal((2 * C, HD)) * 0.1).astype(np.float32),
        "fc0_b": (rng.standard_normal((HD,)) * 0.1).astype(np.float32),
        "fc1_w": (rng.standard_normal((HD, C - 1)) * 0.1).astype(np.float32),
        "steps": 2,
    }
    out = kernel(**inputs)
    print(out.shape, out.dtype)



# revision 8
# speedup vs baseline: 2.3990x; 2.3990x over previous
"""Trainium2 Bass kernel for a 2-step BasicNCA2D cell update (fp8 DoubleRow).

Strategy
--------
Data-parallel over batch: 8 images, one per NeuronCore. Both NCA steps are
fused on-chip (the bf16 state never round-trips to DRAM between steps).

Per step the math is
    y  = depthwise_conv5x5(x, conv_w) + conv_b        (reflect padding)
    h  = relu([x, y] @ fc0_w + fc0_b)
    dx = h @ fc1_w
    x' = concat([x[..., :1], x[..., 1:] + dx])

conv+fc0 are fused into a bank of accumulating matmuls:
    h_pre = sum_{di,dj} x_shift(di,dj) @ M[di,dj],
    M[di,dj] = diag(conv_w[di,dj]) @ fc0_w[24:] (+ fc0_w[:24] at center)

Precision plan (validated numerically, rel err ~1.2e-2 < 2e-2 budget):
the conv+fc0 matmul bank runs in fp8 e4m3 with MatmulPerfMode.DoubleRow
(0.5 cycles/row = 2x PE rate), weights pre-scaled by 2^8 to stay in the
e4m3 normal range and descaled in the relu's activation scale. fc1 runs
in bf16. The carried state x stays in bf16; each stage's quantized fp8
copy is derived from it, so quantization noise does not accumulate.

Layout: rows are processed in 4-row groups, channels zero-padded 24->32
so partition splits land on hardware bases {0,32,64,96}. fp8 inputs live
in one big ring tile [128, S, 516] (partition = 4 rows x 32 ch, slot =
"offset block" m = image rows 4m-2..4m+1, 2 reflect-halo cols per side).
An output group g (rows 4g..4g+3) contracts blocks m=g and m=g+1: the
DoubleRow pair dim of the moving AP strides between the two ring slots
(negative stride at ring wrap), so conv+fc0 for a group is 5 DoubleRow
matmuls (one per horizontal tap) with 2x256 K-rows each, plus one bf16
fc1 matmul: 7 PE passes -> ~2048 PE cycles per 4-row group per step.

Engines: ACT does relu (PSUM->bf16, scale 2^-8, fused bias). DVE does the
single residual add per group (PSUM fp32 + bf16 ring -> bf16 ring; both
stages' adds are single ops because input blocks are offset-aligned while
outputs are group-aligned). Pool (gpsimd) converts bf16 state to the next
stage's fp8 offset blocks (two partition-remap copies per block) and
fills reflect halos. DMA streams fp8+bf16 inputs and bf16 outputs with
pair-wide transfers to bound descriptor-generation time.
"""

import numpy as np
import ml_dtypes

import bass_rust
import concourse.mybir as mybir
import concourse.tile as tile
from concourse import bacc
from concourse.bass_utils import run_bass_kernel_spmd

F32 = mybir.dt.float32
BF16 = mybir.dt.bfloat16
E4 = mybir.dt.float8e4
DRMODE = mybir.MatmulPerfMode.DoubleRow

H = 512
W = 512
C = 24
CP = 32  # padded channels
HD = 32
NCORES = 8
NBLK = H // 4 + 1   # 129 offset blocks per stage
NGRP = H // 4       # 128 output groups per stage
SCALE = 256.0       # fp8 weight pre-scale (power of two)

SLOTS = 16          # ring slots (even, >= pipeline depth * 2 + 4)
XW = 516            # fp8 block width (512 + 2+2 halo)


def _build_nc(steps: int, repeat: int = 1):
    nc = bacc.Bacc("TRN2", target_bir_lowering=False, debug=False)

    # X8: fp8 offset blocks in block-major layout [block, partition, col]
    # (block m = image rows 4m-2..4m+1, partition = 4 rows x 32 ch, halo cols)
    X8 = nc.dram_tensor("X8", [NBLK, 128, XW], E4, kind="ExternalInput")
    # XB: bf16 interior state, group-major [group, partition, col]
    XB = nc.dram_tensor("XB", [NGRP, 128, W], BF16, kind="ExternalInput")
    # WABD[k, dj, half, m]: DoubleRow stationary pairs (A=block m, B=block m+1)
    WABD = nc.dram_tensor("WABD", [128, 5, 2, 128], E4, kind="ExternalInput")
    WC = nc.dram_tensor("WC", [128, 128], BF16, kind="ExternalInput")
    BIAS = nc.dram_tensor("BIAS", [128, 1], F32, kind="ExternalInput")
    Y = nc.dram_tensor("Y", [NGRP, 128, W], BF16, kind="ExternalOutput")

    n_pairs = NGRP // 2  # 64 group-pairs per stage

    with tile.TileContext(nc) as tc:
        with (
            tc.tile_pool(name="wpool", bufs=1) as wpool,
            tc.tile_pool(name="xr", bufs=1) as xrpool,    # fp8 rings (big tiles)
            tc.tile_pool(name="xb", bufs=1) as xbpool,    # bf16 rings (big tiles)
            tc.tile_pool(name="hpool", bufs=3) as hpool,
            tc.tile_pool(name="opool", bufs=4) as opool,
            tc.tile_pool(name="ph", bufs=1, space="PSUM") as ph,
            tc.tile_pool(name="pd", bufs=1, space="PSUM") as pd,
        ):
            wab_t = wpool.tile([128, 5, 2, 128], E4, tag="wab")
            nc.sync.dma_start(wab_t[:], WABD.ap())
            wc_t = wpool.tile([128, 128], BF16, tag="wc")
            nc.sync.dma_start(wc_t[:], WC.ap())
            bias_t = wpool.tile([128, 1], F32, tag="bias")
            nc.sync.dma_start(bias_t[:], BIAS.ap())

            # per-stage rings
            xr = [xrpool.tile([128, SLOTS, XW], E4, tag=f"xr{s}", name=f"xr{s}")
                  for s in range(steps)]
            # bf16 state rings: slot g = image rows 4g..4g+3 (interior cols)
            xbr = [xbpool.tile([128, SLOTS, W], BF16, tag=f"xb{s}", name=f"xb{s}")
                   for s in range(steps)]

            pitch8 = SLOTS * XW

            def load_x0_quad(q):
                """DMA fp8 offset blocks 4q..4q+3 and bf16 groups 4q..4q+3."""
                m = 4 * q
                s = m % SLOTS  # SLOTS % 4 == 0 -> no wrap within a quad
                nc.sync.dma_start(
                    xr[0][:, s : s + 4, :],
                    X8.ap()[m : m + 4].transpose([1, 0, 2]),
                )
                nc.sync.dma_start(
                    xbr[0][:, s : s + 4, :],
                    XB.ap()[m : m + 4].transpose([1, 0, 2]),
                )

            def load_x0_last():
                m = NBLK - 1  # block 128
                s = m % SLOTS
                nc.sync.dma_start(
                    xr[0][:, s : s + 1, :],
                    X8.ap()[m : m + 1].transpose([1, 0, 2]),
                )

            def conv_group(s, g, hp, psl):
                """5 DoubleRow matmuls: blocks m=g (A) and m=g+1 (B)."""
                ring = xr[s]
                sa = g % SLOTS
                sb = (g + 1) % SLOTS
                dslot = sb - sa
                for dj in range(5):
                    mv = ring[:, sa, dj : dj + 512]
                    mvc = mv.copy()
                    mvc.ap = bass_rust.VecI64Pair(
                        [[pitch8, 128], [dslot * XW, 2], [1, 512]]
                    )
                    nc.tensor.matmul(
                        hp[:, psl, :],
                        wab_t[:, dj, :, :],
                        mvc,
                        start=(dj == 0),
                        stop=(dj == 4),
                        perf_mode=DRMODE,
                    )

            pend = [dict() for _ in range(steps)]

            def stage_part1(s, t):
                """conv + relu for stage s, group pair t (groups 2t, 2t+1)."""
                hp = ph.tile([128, 2, 512], F32, tag=f"hp{s}", name=f"hp{s}_{t}")
                conv_group(s, 2 * t, hp, 0)
                conv_group(s, 2 * t + 1, hp, 1)
                h = hpool.tile([128, 2, 512], BF16, tag=f"h{s}", name=f"h{s}_{t}")
                nc.scalar.activation(
                    h[:], hp[:], mybir.ActivationFunctionType.Relu,
                    bias=bias_t[:], scale=1.0 / SCALE,
                )
                pend[s][t] = h

            out_quad = {}

            def stage_part2(s, t):
                """fc1 + residual add for stage s, pair t (one iter later)."""
                last = s == steps - 1
                h = pend[s].pop(t)
                dxp = pd.tile([128, 2, 512], F32, tag=f"dx{s}", name=f"dx{s}_{t}")
                nc.tensor.matmul(dxp[:, 0, :], wc_t[:], h[:, 0, :], start=True, stop=True)
                nc.tensor.matmul(dxp[:, 1, :], wc_t[:], h[:, 1, :], start=True, stop=True)
                g = 2 * t
                sg = g % SLOTS  # even => sg+1 in range, no wrap
                src = xbr[s][:, sg : sg + 2, :]
                if last:
                    if t % 2 == 0:
                        out_quad[t // 2] = opool.tile(
                            [128, 4, 512], BF16, tag="out", name=f"out_{t // 2}"
                        )
                    out = out_quad[t // 2]
                    j = (t % 2) * 2
                    nc.vector.tensor_add(out[:, j : j + 2, :], dxp[:], src)
                    if t % 2 == 1:
                        nc.sync.dma_start(
                            Y.ap()[2 * t - 2 : 2 * t + 2].transpose([1, 0, 2]),
                            out_quad.pop(t // 2)[:],
                        )
                else:
                    dst = xbr[s + 1][:, sg : sg + 2, :]
                    nc.vector.tensor_add(dst, dxp[:], src)

            def cvt_pair(s, c):
                """Build fp8 offset blocks 2c,2c+1 of stage s from bf16 ring."""
                for m in (2 * c, 2 * c + 1):
                    cvt_block(s, m)

            def cvt_block(s, m):
                """fp8 offset block m (rows 4m-2..4m+1) from group-aligned bf16."""
                ring = xr[s]
                xbsrc = xbr[s]
                sm = m % SLOTS
                dst = ring[:, sm, :]
                if m == 0:
                    # rows -2,-1 are reflect rows (image 2, 1); rows 0,1 real
                    nc.gpsimd.tensor_copy(dst[0:32, 2:514], xbsrc[64:96, (0) % SLOTS, :])
                    nc.gpsimd.tensor_copy(dst[32:64, 2:514], xbsrc[32:64, (0) % SLOTS, :])
                    nc.gpsimd.tensor_copy(dst[64:128, 2:514], xbsrc[0:64, (0) % SLOTS, :])
                elif m == NBLK - 1:
                    # rows 510,511 real; rows 512,513 reflect (image 510, 509)
                    gprev = NGRP - 1  # 127
                    sp = gprev % SLOTS
                    nc.gpsimd.tensor_copy(dst[0:64, 2:514], xbsrc[64:128, sp, :])
                    nc.gpsimd.tensor_copy(dst[64:96, 2:514], xbsrc[64:96, sp, :])
                    nc.gpsimd.tensor_copy(dst[96:128, 2:514], xbsrc[32:64, sp, :])
                else:
                    sa = (m - 1) % SLOTS
                    sb = m % SLOTS
                    nc.gpsimd.tensor_copy(dst[0:64, 2:514], xbsrc[64:128, sa, :])
                    nc.gpsimd.tensor_copy(dst[64:128, 2:514], xbsrc[0:64, sb, :])
                # reflect halo cols (image cols 2,1 / 510,509 at tile 4,3 / 512,511)
                for vc, pc in ((0, 4), (1, 3), (514, 512), (515, 511)):
                    nc.gpsimd.tensor_copy(dst[:, vc : vc + 1], dst[:, pc : pc + 1])

            # software pipeline over group pairs
            L1, L1B, LCV, L2, L2B = 2, 3, 4, 6, 7
            n_iters = n_pairs + L2B + 1
            for _rep in range(repeat):
                for i in range(n_iters):
                    if i % 2 == 0 and i // 2 < n_pairs // 2:
                        load_x0_quad(i // 2)
                    if i == n_pairs:
                        load_x0_last()
                    p = i - L1
                    if 0 <= p < n_pairs:
                        stage_part1(0, p)
                    p = i - L1B
                    if 0 <= p < n_pairs:
                        stage_part2(0, p)
                    if steps > 1:
                        c = i - LCV
                        if 0 <= c < n_pairs:
                            cvt_pair(1, c)
                        if c == n_pairs:
                            cvt_block(1, NBLK - 1)
                        p = i - L2
                        if 0 <= p < n_pairs:
                            stage_part1(1, p)
                        p = i - L2B
                        if 0 <= p < n_pairs:
                            stage_part2(1, p)

    nc.compile()
    return nc


_NC_CACHE = {}
_REPEAT = 1


def _get_nc(steps):
    key = (steps, _REPEAT)
    if key not in _NC_CACHE:
        _NC_CACHE[key] = _build_nc(steps, repeat=_REPEAT)
    return _NC_CACHE[key]


def _prep_weights(conv_w, conv_b, fc0_w, fc0_b, fc1_w):
    conv_w = np.asarray(conv_w, np.float64)[:, :, 0, :]  # [5,5,24]
    W1 = np.asarray(fc0_w, np.float64)[:C]  # [24,32]
    W2 = np.asarray(fc0_w, np.float64)[C:]  # [24,32]
    fc1_w = np.asarray(fc1_w, np.float64)  # [32,23]

    # M[ki, kj] = diag(conv_w[ki,kj]) @ W2 (+ W1 at center)
    M = conv_w[:, :, :, None] * W2[None, None, :, :]  # [5,5,24,32]
    M[2, 2] += W1

    WAB = np.zeros((2, 5, 128, 128), np.float64)
    for dj in range(5):
        for g in range(4):
            for f in range(4):
                ka = g - f  # di+2 for block A (di = g-f-2)
                if g >= f and 0 <= ka <= 4:
                    WAB[0, dj, g * 32 : g * 32 + C, f * 32 : f * 32 + HD] = M[ka, dj]
                kb = g + 4 - f  # di+2 for block B (di = g-f+2)
                if g <= f and 0 <= kb <= 4:
                    WAB[1, dj, g * 32 : g * 32 + C, f * 32 : f * 32 + HD] = M[kb, dj]

    # DoubleRow layout: [K, dj, half, M], pre-scaled, e4m3
    WABD = np.ascontiguousarray(
        (WAB * SCALE).transpose(2, 1, 0, 3)
    ).astype(np.float32).astype(ml_dtypes.float8_e4m3fn)

    WCm = np.zeros((128, 128), np.float32)
    for f in range(4):
        WCm[f * 32 : f * 32 + HD, f * 32 + 1 : f * 32 + C] = fc1_w
    WCm = WCm.astype(ml_dtypes.bfloat16)

    bias_eff = (
        np.asarray(fc0_b, np.float64) + np.asarray(conv_b, np.float64) @ W2
    ).astype(np.float32)
    BIAS = np.tile(bias_eff, 4).reshape(128, 1)
    return WABD, WCm, BIAS


def _run_pass(x_chw, WABD, WCm, BIAS, steps):
    """One device invocation: `steps` NCA steps on x [B, C, H, W] fp32."""
    B = x_chw.shape[0]
    xpad = np.pad(x_chw, ((0, 0), (0, 0), (2, 2), (2, 2)), mode="reflect")
    x8 = np.zeros((B, CP, H + 4, XW), np.float32)
    x8[:, :C] = xpad
    # block-major fp8: [B, NBLK, 128, XW], block m = padded rows 4m..4m+3
    X8 = np.ascontiguousarray(
        x8.reshape(B, CP, NBLK, 4, XW).transpose(0, 2, 3, 1, 4)
        .reshape(B, NBLK, 128, XW)
    ).astype(ml_dtypes.float8_e4m3fn)
    xb = np.zeros((B, CP, H, W), np.float32)
    xb[:, :C] = x_chw
    # group-major bf16: [B, NGRP, 128, W]
    XB = np.ascontiguousarray(
        xb.reshape(B, CP, NGRP, 4, W).transpose(0, 2, 3, 1, 4)
        .reshape(B, NGRP, 128, W)
    ).astype(ml_dtypes.bfloat16)
    nc = _get_nc(steps)
    in_maps = [
        {"X8": X8[i % B], "XB": XB[i % B], "WABD": WABD, "WC": WCm, "BIAS": BIAS}
        for i in range(NCORES)
    ]
    res = run_bass_kernel_spmd(nc, in_maps, core_ids=list(range(NCORES)))
    globals()["LAST_RESULTS"] = res
    # Y: [NGRP, 128, W] group-major bf16 -> [C, H, W] fp32
    out = np.stack(
        [
            res.results[i]["Y"].astype(np.float32)
            .reshape(NGRP, 4, CP, W).transpose(2, 0, 1, 3).reshape(CP, H, W)[:C]
            for i in range(B)
        ]
    )  # [B, C, H, W]
    return out


def kernel(x, conv_w, conv_b, fc0_w, fc0_b, fc1_w, steps):
    steps = int(steps)
    x = np.asarray(x, np.float32)
    B = x.shape[0]
    assert x.shape == (B, H, W, C) and 1 <= B <= NCORES, x.shape
    if steps <= 0:
        return x.copy()

    WABD, WCm, BIAS = _prep_weights(conv_w, conv_b, fc0_w, fc0_b, fc1_w)
    x_chw = np.ascontiguousarray(x.transpose(0, 3, 1, 2))
    # device pipeline supports 2 fused steps; decompose larger step counts
    while steps > 0:
        n = 2 if steps >= 2 else 1
        x_chw = _run_pass(x_chw, WABD, WCm, BIAS, n)
        steps -= n
    return np.ascontiguousarray(x_chw.transpose(0, 2, 3, 1)).astype(np.float32)


if __name__ == "__main__":
    rng = np.random.default_rng(0)
    inputs = {
        "x": rng.standard_normal((8, H, W, C), dtype=np.float32),
        "conv_w": (rng.standard_normal((5, 5, 1, C)) * 0.1).astype(np.float32),
        "conv_b": (rng.standard_normal((C,)) * 0.1).astype(np.float32),
        "fc0_w": (rng.standard_normal((2 * C, HD)) * 0.1).astype(np.float32),
        "fc0_b": (rng.standard_normal((HD,)) * 0.1).astype(np.float32),
        "fc1_w": (rng.standard_normal((HD, C - 1)) * 0.1).astype(np.float32),
        "steps": 2,
    }
    out = kernel(**inputs)
    print(out.shape, out.dtype)


# revision 16
# speedup vs baseline: 3.1169x; 1.2992x over previous
"""Trainium2 Bass kernel for a 2-step BasicNCA2D cell update (fp8 DoubleRow).

Strategy
--------
Data-parallel over batch: 8 images, one per NeuronCore. Both NCA steps are
fused on-chip (the bf16 state never round-trips to DRAM between steps).

Per step the math is
    y  = depthwise_conv5x5(x, conv_w) + conv_b        (reflect padding)
    h  = relu([x, y] @ fc0_w + fc0_b)
    dx = h @ fc1_w
    x' = concat([x[..., :1], x[..., 1:] + dx])

conv+fc0 are fused into a bank of accumulating matmuls:
    h_pre = sum_{di,dj} x_shift(di,dj) @ M[di,dj],
    M[di,dj] = diag(conv_w[di,dj]) @ fc0_w[24:] (+ fc0_w[:24] at center)

Precision plan (validated numerically, rel err ~1.2e-2 < 2e-2 budget):
the conv+fc0 matmul bank runs in fp8 e4m3 with MatmulPerfMode.DoubleRow
(0.5 cycles/row = 2x PE rate), weights pre-scaled by 2^8 to stay in the
e4m3 normal range and descaled in the relu's activation scale. fc1 runs
in bf16. The carried state x stays in bf16; each stage's quantized fp8
copy is derived from it, so quantization noise does not accumulate.

Layout: rows are processed in 4-row groups, channels zero-padded 24->32
so partition splits land on hardware bases {0,32,64,96}. fp8 inputs live
in one big ring tile [128, S, 516] (partition = 4 rows x 32 ch, slot =
"offset block" m = image rows 4m-2..4m+1, 2 reflect-halo cols per side).
An output group g (rows 4g..4g+3) contracts blocks m=g and m=g+1: the
DoubleRow pair dim of the moving AP strides between the two ring slots
(negative stride at ring wrap), so conv+fc0 for a group is 5 DoubleRow
matmuls (one per horizontal tap) with 2x256 K-rows each, plus one bf16
fc1 matmul: 7 PE passes -> ~2048 PE cycles per 4-row group per step.

Engines: ACT does relu (PSUM->bf16, scale 2^-8, fused bias). DVE does the
single residual add per group (PSUM fp32 + bf16 ring -> bf16 ring; both
stages' adds are single ops because input blocks are offset-aligned while
outputs are group-aligned). Pool (gpsimd) converts bf16 state to the next
stage's fp8 offset blocks (two partition-remap copies per block) and
fills reflect halos. DMA streams fp8+bf16 inputs and bf16 outputs with
pair-wide transfers to bound descriptor-generation time.
"""

import numpy as np
import ml_dtypes

import bass_rust
import concourse.mybir as mybir
import concourse.tile as tile
from concourse import bacc
from concourse.bass_utils import run_bass_kernel_spmd

F32 = mybir.dt.float32
BF16 = mybir.dt.bfloat16
E4 = mybir.dt.float8e4
DRMODE = mybir.MatmulPerfMode.DoubleRow

H = 512
W = 512
C = 24
CP = 32  # padded channels
HD = 32
NCORES = 8
NBLK = H // 4 + 1   # 129 offset blocks per stage
NGRP = H // 4       # 128 output groups per stage
SCALE = 256.0       # fp8 weight pre-scale (power of two)

SLOTS = 16          # ring slots (even, >= pipeline depth * 2 + 4)
XW = 516            # fp8 block width (512 + 2+2 halo)


def _build_nc(steps: int, repeat: int = 1):
    nc = bacc.Bacc("TRN2", target_bir_lowering=False, debug=False)

    # X8: fp8 offset blocks in block-major layout [block, partition, col]
    # (block m = image rows 4m-2..4m+1, partition = 4 rows x 32 ch, halo cols)
    X8 = nc.dram_tensor("X8", [NBLK, 128, XW], E4, kind="ExternalInput")
    # XB: bf16 interior state, group-major [group, partition, col]
    XB = nc.dram_tensor("XB", [NGRP, 128, W], BF16, kind="ExternalInput")
    # WABD[k, dj, half, m]: DoubleRow stationary pairs (A=block m, B=block m+1)
    WABD = nc.dram_tensor("WABD", [128, 5, 2, 128], E4, kind="ExternalInput")
    # WC[k, half, m]: fp8 DoubleRow fc1 stationary; half 1 is zero
    WC = nc.dram_tensor("WC", [128, 2, 128], E4, kind="ExternalInput")
    BIAS = nc.dram_tensor("BIAS", [128, 1], F32, kind="ExternalInput")
    Y = nc.dram_tensor("Y", [NGRP, 128, W], BF16, kind="ExternalOutput")

    n_pairs = NGRP // 2  # 64 group-pairs per stage

    with tile.TileContext(nc) as tc:
        with (
            tc.tile_pool(name="wpool", bufs=1) as wpool,
            tc.tile_pool(name="xr", bufs=1) as xrpool,    # fp8 rings (big tiles)
            tc.tile_pool(name="xb", bufs=1) as xbpool,    # bf16 rings (big tiles)
            tc.tile_pool(name="hpool", bufs=3) as hpool,
            tc.tile_pool(name="opool", bufs=4) as opool,
            tc.tile_pool(name="ph", bufs=1, space="PSUM") as ph,
            tc.tile_pool(name="pd", bufs=1, space="PSUM") as pd,
        ):
            wab_t = wpool.tile([128, 5, 2, 128], E4, tag="wab")
            nc.sync.dma_start(wab_t[:], WABD.ap())
            wc_t = wpool.tile([128, 2, 128], E4, tag="wc")
            nc.sync.dma_start(wc_t[:], WC.ap())
            bias_t = wpool.tile([128, 1], F32, tag="bias")
            nc.sync.dma_start(bias_t[:], BIAS.ap())

            # per-stage rings
            xr = [xrpool.tile([128, SLOTS, XW], E4, tag=f"xr{s}", name=f"xr{s}")
                  for s in range(steps)]
            # bf16 state rings: slot g = image rows 4g..4g+3 (interior cols)
            xbr = [xbpool.tile([128, SLOTS, W], BF16, tag=f"xb{s}", name=f"xb{s}")
                   for s in range(steps)]

            pitch8 = SLOTS * XW

            def load_x0_quad(q):
                """DMA fp8 offset blocks 4q..4q+3 and bf16 groups 4q..4q+3."""
                m = 4 * q
                s = m % SLOTS  # SLOTS % 4 == 0 -> no wrap within a quad
                nc.sync.dma_start(
                    xr[0][:, s : s + 4, :],
                    X8.ap()[m : m + 4].transpose([1, 0, 2]),
                )
                nc.sync.dma_start(
                    xbr[0][:, s : s + 4, :],
                    XB.ap()[m : m + 4].transpose([1, 0, 2]),
                )

            def load_x0_last():
                m = NBLK - 1  # block 128
                s = m % SLOTS
                nc.sync.dma_start(
                    xr[0][:, s : s + 1, :],
                    X8.ap()[m : m + 1].transpose([1, 0, 2]),
                )

            def conv_group(s, g, hp, psl):
                """5 DoubleRow matmuls: blocks m=g (A) and m=g+1 (B)."""
                ring = xr[s]
                sa = g % SLOTS
                sb = (g + 1) % SLOTS
                dslot = sb - sa
                for dj in range(5):
                    mv = ring[:, sa, dj : dj + 512]
                    mvc = mv.copy()
                    mvc.ap = bass_rust.VecI64Pair(
                        [[pitch8, 128], [dslot * XW, 2], [1, 512]]
                    )
                    nc.tensor.matmul(
                        hp[:, psl, :],
                        wab_t[:, dj, :, :],
                        mvc,
                        start=(dj == 0),
                        stop=(dj == 4),
                        perf_mode=DRMODE,
                    )

            pend = [dict() for _ in range(steps)]

            def stage_part1(s, t):
                """conv + relu for stage s, group pair t (groups 2t, 2t+1)."""
                hp = ph.tile([128, 2, 512], F32, tag=f"hp{s}", name=f"hp{s}_{t}")
                conv_group(s, 2 * t, hp, 0)
                conv_group(s, 2 * t + 1, hp, 1)
                h = hpool.tile([128, 2, 512], E4, tag=f"h{s}", name=f"h{s}_{t}")
                nc.scalar.activation(
                    h[:], hp[:], mybir.ActivationFunctionType.Relu,
                    bias=bias_t[:], scale=1.0 / SCALE,
                )
                pend[s][t] = h

            out_quad = {}

            def stage_part2(s, t):
                """fc1 + residual add for stage s, pair t (one iter later)."""
                last = s == steps - 1
                h = pend[s].pop(t)
                dxp = pd.tile([128, 2, 512], F32, tag=f"dx{s}", name=f"dx{s}_{t}")
                for j in range(2):
                    # fp8 DoubleRow fc1: half0 = (WC, h_j), half1 = (0, h_1-j)
                    mv = h[:, j, :]
                    mvc = mv.copy()
                    mvc.ap = bass_rust.VecI64Pair(
                        [[1024, 128], [(1 - 2 * j) * 512, 2], [1, 512]]
                    )
                    nc.tensor.matmul(
                        dxp[:, j, :], wc_t[:], mvc,
                        start=True, stop=True, perf_mode=DRMODE,
                    )
                g = 2 * t
                sg = g % SLOTS  # even => sg+1 in range, no wrap
                src = xbr[s][:, sg : sg + 2, :]
                if last:
                    if t % 2 == 0:
                        out_quad[t // 2] = opool.tile(
                            [128, 4, 512], BF16, tag="out", name=f"out_{t // 2}"
                        )
                    out = out_quad[t // 2]
                    j = (t % 2) * 2
                    nc.vector.tensor_add(out[:, j : j + 2, :], dxp[:], src)
                    if t % 2 == 1:
                        nc.sync.dma_start(
                            Y.ap()[2 * t - 2 : 2 * t + 2].transpose([1, 0, 2]),
                            out_quad.pop(t // 2)[:],
                        )
                else:
                    dst = xbr[s + 1][:, sg : sg + 2, :]
                    nc.vector.tensor_add(dst, dxp[:], src)

            def halo_op(s, sl, nslot):
                """Reflect halo cols for `nslot` ring slots starting at sl."""
                ring = xr[s]
                dst = ring[:, sl : sl + nslot, 0:2]
                d = dst.copy()
                d.ap = bass_rust.VecI64Pair(
                    [[pitch8, 128], [XW, nslot], [514, 2], [1, 2]]
                )
                src = ring[:, sl : sl + nslot, 4:6]
                sc = src.copy()
                sc.ap = bass_rust.VecI64Pair(
                    [[pitch8, 128], [XW, nslot], [508, 2], [-1, 2]]
                )
                nc.gpsimd.tensor_copy(d, sc)

            def cvt_pair(s, c):
                """Build fp8 offset blocks 2c,2c+1 of stage s from bf16 ring.

                Block m: partitions 0:64 = rows 4m-2,4m-1 (bf16 group m-1,
                partitions 64:128), partitions 64:128 = rows 4m,4m+1 (bf16
                group m, partitions 0:64)."""
                ring = xr[s]
                xbsrc = xbr[s]
                m = 2 * c
                sl = m % SLOTS
                if c == 0:
                    # block 0: rows -2,-1 are reflect rows (image 2, 1)
                    nc.gpsimd.tensor_copy(ring[0:32, 0, 2:514], xbsrc[64:96, 0, :])
                    nc.gpsimd.tensor_copy(ring[32:64, 0, 2:514], xbsrc[32:64, 0, :])
                    nc.gpsimd.tensor_copy(ring[64:128, 0, 2:514], xbsrc[0:64, 0, :])
                    nc.gpsimd.tensor_copy(ring[0:64, 1, 2:514], xbsrc[64:128, 0, :])
                    nc.scalar.copy(ring[64:128, 1, 2:514], xbsrc[0:64, 1, :])
                else:
                    sprev = (m - 1) % SLOTS
                    lo = xbsrc[64:128, sprev, :].copy()
                    lo.ap = bass_rust.VecI64Pair(
                        [[SLOTS * W, 64], [(sl - sprev) * W, 2], [1, W]]
                    )
                    nc.gpsimd.tensor_copy(ring[0:64, sl : sl + 2, 2:514], lo)
                    hi_dst = ring[64:128, sl : sl + 2, 2:514]
                    hi_src = xbsrc[0:64, sl : sl + 2, :]
                    r = c % 8
                    if r in (0, 2, 4):
                        nc.scalar.copy(hi_dst, hi_src)
                    elif r == 6:
                        nc.vector.tensor_copy(hi_dst, hi_src)
                    else:
                        nc.gpsimd.tensor_copy(hi_dst, hi_src)
                halo_op(s, sl, 2)

            def cvt_last(s):
                """fp8 block 128: rows 510,511 real; 512,513 reflect (510,509)."""
                ring = xr[s]
                xbsrc = xbr[s]
                m = NBLK - 1
                sl = m % SLOTS
                sp = (NGRP - 1) % SLOTS
                nc.gpsimd.tensor_copy(ring[0:64, sl, 2:514], xbsrc[64:128, sp, :])
                nc.gpsimd.tensor_copy(ring[64:96, sl, 2:514], xbsrc[64:96, sp, :])
                nc.gpsimd.tensor_copy(ring[96:128, sl, 2:514], xbsrc[32:64, sp, :])
                halo_op(s, sl, 1)

            # software pipeline over group pairs
            L1, L1B, LCV, L2, L2B = 2, 3, 4, 6, 7
            n_iters = n_pairs + L2B + 1
            for _rep in range(repeat):
                for i in range(n_iters):
                    if i % 2 == 0 and i // 2 < n_pairs // 2:
                        load_x0_quad(i // 2)
                    if i == n_pairs:
                        load_x0_last()
                    p = i - L1
                    if 0 <= p < n_pairs:
                        stage_part1(0, p)
                    p = i - L1B
                    if 0 <= p < n_pairs:
                        stage_part2(0, p)
                    if steps > 1:
                        c = i - LCV
                        if 0 <= c < n_pairs:
                            cvt_pair(1, c)
                        if c == n_pairs:
                            cvt_last(1)
                        p = i - L2
                        if 0 <= p < n_pairs:
                            stage_part1(1, p)
                        p = i - L2B
                        if 0 <= p < n_pairs:
                            stage_part2(1, p)

    nc.compile()
    return nc


_NC_CACHE = {}
_REPEAT = 1


def _get_nc(steps):
    key = (steps, _REPEAT)
    if key not in _NC_CACHE:
        _NC_CACHE[key] = _build_nc(steps, repeat=_REPEAT)
    return _NC_CACHE[key]


def _prep_weights(conv_w, conv_b, fc0_w, fc0_b, fc1_w):
    conv_w = np.asarray(conv_w, np.float64)[:, :, 0, :]  # [5,5,24]
    W1 = np.asarray(fc0_w, np.float64)[:C]  # [24,32]
    W2 = np.asarray(fc0_w, np.float64)[C:]  # [24,32]
    fc1_w = np.asarray(fc1_w, np.float64)  # [32,23]

    # M[ki, kj] = diag(conv_w[ki,kj]) @ W2 (+ W1 at center)
    M = conv_w[:, :, :, None] * W2[None, None, :, :]  # [5,5,24,32]
    M[2, 2] += W1

    WAB = np.zeros((2, 5, 128, 128), np.float64)
    for dj in range(5):
        for g in range(4):
            for f in range(4):
                ka = g - f  # di+2 for block A (di = g-f-2)
                if g >= f and 0 <= ka <= 4:
                    WAB[0, dj, g * 32 : g * 32 + C, f * 32 : f * 32 + HD] = M[ka, dj]
                kb = g + 4 - f  # di+2 for block B (di = g-f+2)
                if g <= f and 0 <= kb <= 4:
                    WAB[1, dj, g * 32 : g * 32 + C, f * 32 : f * 32 + HD] = M[kb, dj]

    # DoubleRow layout: [K, dj, half, M], pre-scaled, e4m3
    WABD = np.ascontiguousarray(
        (WAB * SCALE).transpose(2, 1, 0, 3)
    ).astype(np.float32).astype(ml_dtypes.float8_e4m3fn)

    WCm = np.zeros((128, 2, 128), np.float32)
    for f in range(4):
        WCm[f * 32 : f * 32 + HD, 0, f * 32 + 1 : f * 32 + C] = fc1_w
    WCm = WCm.astype(ml_dtypes.float8_e4m3fn)

    bias_eff = (
        np.asarray(fc0_b, np.float64) + np.asarray(conv_b, np.float64) @ W2
    ).astype(np.float32)
    BIAS = np.tile(bias_eff, 4).reshape(128, 1)
    return WABD, WCm, BIAS


def _run_pass(x_chw, WABD, WCm, BIAS, steps):
    """One device invocation: `steps` NCA steps on x [B, C, H, W] fp32."""
    B = x_chw.shape[0]
    xpad = np.pad(x_chw, ((0, 0), (0, 0), (2, 2), (2, 2)), mode="reflect")
    x8 = np.zeros((B, CP, H + 4, XW), np.float32)
    x8[:, :C] = xpad
    # block-major fp8: [B, NBLK, 128, XW], block m = padded rows 4m..4m+3
    X8 = np.ascontiguousarray(
        x8.reshape(B, CP, NBLK, 4, XW).transpose(0, 2, 3, 1, 4)
        .reshape(B, NBLK, 128, XW)
    ).astype(ml_dtypes.float8_e4m3fn)
    xb = np.zeros((B, CP, H, W), np.float32)
    xb[:, :C] = x_chw
    # group-major bf16: [B, NGRP, 128, W]
    XB = np.ascontiguousarray(
        xb.reshape(B, CP, NGRP, 4, W).transpose(0, 2, 3, 1, 4)
        .reshape(B, NGRP, 128, W)
    ).astype(ml_dtypes.bfloat16)
    nc = _get_nc(steps)
    in_maps = [
        {"X8": X8[i % B], "XB": XB[i % B], "WABD": WABD, "WC": WCm, "BIAS": BIAS}
        for i in range(NCORES)
    ]
    res = run_bass_kernel_spmd(nc, in_maps, core_ids=list(range(NCORES)))
    globals()["LAST_RESULTS"] = res
    # Y: [NGRP, 128, W] group-major bf16 -> [C, H, W] fp32
    out = np.stack(
        [
            res.results[i]["Y"].astype(np.float32)
            .reshape(NGRP, 4, CP, W).transpose(2, 0, 1, 3).reshape(CP, H, W)[:C]
            for i in range(B)
        ]
    )  # [B, C, H, W]
    return out


def kernel(x, conv_w, conv_b, fc0_w, fc0_b, fc1_w, steps):
    steps = int(steps)
    x = np.asarray(x, np.float32)
    B = x.shape[0]
    assert x.shape == (B, H, W, C) and 1 <= B <= NCORES, x.shape
    if steps <= 0:
        return x.copy()

    WABD, WCm, BIAS = _prep_weights(conv_w, conv_b, fc0_w, fc0_b, fc1_w)
    x_chw = np.ascontiguousarray(x.transpose(0, 3, 1, 2))
    # device pipeline supports 2 fused steps; decompose larger step counts
    while steps > 0:
        n = 2 if steps >= 2 else 1
        x_chw = _run_pass(x_chw, WABD, WCm, BIAS, n)
        steps -= n
    return np.ascontiguousarray(x_chw.transpose(0, 2, 3, 1)).astype(np.float32)


if __name__ == "__main__":
    rng = np.random.default_rng(0)
    inputs = {
        "x": rng.standard_normal((8, H, W, C), dtype=np.float32),
        "conv_w": (rng.standard_normal((5, 5, 1, C)) * 0.1).astype(np.float32),
        "conv_b": (rng.standard_normal((C,)) * 0.1).astype(np.float32),
        "fc0_w": (rng.standard_normal((2 * C, HD)) * 0.1).astype(np.float32),
        "fc0_b": (rng.standard_normal((HD,)) * 0.1).astype(np.float32),
        "fc1_w": (rng.standard_normal((HD, C - 1)) * 0.1).astype(np.float32),
        "steps": 2,
    }
    out = kernel(**inputs)
    print(out.shape, out.dtype)
